# revision 67
# baseline (speedup 1.0000x reference)
"""Trainium2 Bass kernel for nn_DenseRelativeLoc.

Data-parallel over batch: 32 images per NeuronCore x 8 cores.

Key idea vs the projection+one-hot baseline: the point sampling is done by
the DMA engines (dma_gather transpose mode), not the PE. Host stores x as
row-major [BPC*196, 768] bf16 in DRAM; on-device int16 row indices
(196*b + 14*px0 + px1) drive a transposed gather that lands features
directly in matmul-ready layout [128 c_in, 6 c_chunk, n_samples]. The PE
then only runs dense GEMMs:

    h1T = relu(W1a^T fx + W1b^T fy + b1)   [512, 392/pair]  (12 K-chunks)
    h2T = relu(W2^T h1T + b2)              [512, 392]
    predT = W3^T h2T + b3                  [2, 392]

All matmuls bf16 with fp32 PSUM accumulation, N=392 streams (no partition
waste, weight loads hidden under streams). Gather chunks start at even-pair
sample offsets (multiple of 16 for the wrapped idx layout) and overlap so
every pair lives contiguously in one chunk tile.

deltaxy = float(pxs - pys) + 13 computed on-device with DVE ops.
"""
import sys
import types

import numpy as np

B, C, H, W_IMG = 256, 768, 14, 14
S = 196          # sample count == H*W
HID = 512
OUT = 2
N_CORES = 8
BPC = B // N_CORES      # batches per core
NP = BPC * S            # 6272 samples per core
S2 = 2 * S              # samples per pair of batches (392)
PAIRS = BPC // 2        # 16
KC1 = 2 * C // 128      # 12 contraction chunks for GEMM1
MJ = HID // 128         # 4 HID chunks
SPAD = 224              # samples per batch padded to 14*16 in the idx stream
FPB = SPAD * BPC        # padded idx stream length (7168 = 16*448)
NIW = 2 * FPB // 16     # wrapped idx columns (896): per-pair x+y blocks
PCH = 2 * SPAD          # columns per pair per branch block (448)


def _install_ntff_hook():
    try:
        import antenv.axon_hooks  # noqa: F401
        return
    except ImportError:
        pass
    try:
        from trn_agent_boot.trn_boot import _ntff_profile_via_ctypes
        hook = _ntff_profile_via_ctypes("/opt/axon/libaxon_pjrt.so")
    except Exception:
        hook = None
    mod = types.ModuleType("antenv.axon_hooks")
    mod.get_axon_ntff_profile_hook = lambda: hook
    sys.modules["antenv.axon_hooks"] = mod


def _build_nc():
    from contextlib import ExitStack

    import concourse.bass as bass
    import concourse.bacc as bacc
    import concourse.mybir as mybir
    import concourse.tile as tile

    dt = mybir.dt
    f32, bf16, i32, i16 = dt.float32, dt.bfloat16, dt.int32, dt.int16
    AF = mybir.ActivationFunctionType
    ALU = mybir.AluOpType

    nc = bacc.Bacc(None, target_bir_lowering=False)

    # x host-prepped as [NP, C] bf16 row-major (one 1536B row per (b, hw))
    x_t = nc.dram_tensor("x", [NP, C], bf16, kind="ExternalInput")
    pxs_t = nc.dram_tensor("pxs", [BPC, S2], i32, kind="ExternalInput")
    pys_t = nc.dram_tensor("pys", [BPC, S2], i32, kind="ExternalInput")
    w1_t = nc.dram_tensor("W1", [2 * C, HID], bf16, kind="ExternalInput")
    w2_t = nc.dram_tensor("W2", [HID, HID], bf16, kind="ExternalInput")
    w3_t = nc.dram_tensor("W3", [HID, OUT], bf16, kind="ExternalInput")
    b1_t = nc.dram_tensor("b1", [HID], f32, kind="ExternalInput")
    b2_t = nc.dram_tensor("b2", [HID], f32, kind="ExternalInput")
    b3_t = nc.dram_tensor("b3", [OUT], f32, kind="ExternalInput")
    boff_t = nc.dram_tensor("boff", [BPC], f32, kind="ExternalInput")
    # predT: [2, NP] (component-major); host transposes after gather
    pred_t = nc.dram_tensor("predT", [OUT, NP], f32, kind="ExternalOutput")
    delta_t = nc.dram_tensor("deltaxy", [NP, OUT], f32, kind="ExternalOutput")

    with ExitStack() as ctx:
        tc = ctx.enter_context(tile.TileContext(nc))
        wpool = ctx.enter_context(tc.tile_pool(name="w", bufs=1))
        fbpool = ctx.enter_context(tc.tile_pool(name="fb", bufs=3))
        h1pool = ctx.enter_context(tc.tile_pool(name="h1", bufs=2))
        h2pool = ctx.enter_context(tc.tile_pool(name="h2", bufs=2))
        opool = ctx.enter_context(tc.tile_pool(name="op", bufs=2))
        idxpool = ctx.enter_context(tc.tile_pool(name="idx", bufs=1))
        ps1 = ctx.enter_context(tc.tile_pool(name="ps1", bufs=1, space="PSUM"))
        ps2 = ctx.enter_context(tc.tile_pool(name="ps2", bufs=1, space="PSUM"))
        dram = ctx.enter_context(tc.tile_pool(name="dram", bufs=1, space="DRAM"))

        # ---------------- index prep (feeds the gathers) --------------------
        # per-batch idx streams padded to SPAD=224 (a multiple of 16) so the
        # wrapped layout dma_gather wants is expressible with large-run DMAs:
        # replicate 8x in SBUF -> one contiguous write to DRAM -> one XBAR
        # transpose back as [128, NIW]
        pxs_sb = idxpool.tile([BPC, 2 * SPAD], i32, name="pxs_sb", tag="pxs_sb")
        nc.vector.memset(pxs_sb[:], 0)
        nc.sync.dma_start(pxs_sb[:, 0:S2], pxs_t[:, :])
        pys_sb = idxpool.tile([BPC, 2 * SPAD], i32, name="pys_sb", tag="pys_sb")
        nc.vector.memset(pys_sb[:], 0)
        nc.sync.dma_start(pys_sb[:, 0:S2], pys_t[:, :])
        boff_sb = idxpool.tile([BPC, 1], f32, name="boff_sb", tag="boff_sb")
        nc.sync.dma_start(boff_sb[:], boff_t[0:BPC])
        ones_row = wpool.tile([1, S2], bf16, name="ones_row", tag="ones_row")
        nc.vector.memset(ones_row[:], 1.0)

        d = dram.tile([16 * FPB], i16, name="idxd", tag="idxd")
        doff = d[:].offset
        for nm, src in (("x", pxs_sb), ("y", pys_sb)):
            tmp = idxpool.tile([BPC, SPAD], i32, name=f"itmp{nm}", tag=f"itmp{nm}")
            nc.vector.tensor_scalar(
                tmp[:], src[:, 0:2 * SPAD:2], float(W_IMG), None, op0=ALU.mult
            )
            t2 = idxpool.tile([BPC, SPAD], i32, name=f"iadd{nm}", tag=f"iadd{nm}")
            nc.vector.tensor_tensor(t2[:], tmp[:], src[:, 1:2 * SPAD:2], ALU.add)
            t3 = idxpool.tile([BPC, SPAD], f32, name=f"iglob{nm}", tag=f"iglob{nm}")
            nc.vector.tensor_scalar(
                t3[:], t2[:], boff_sb[:], None, op0=ALU.add
            )
            w16 = idxpool.tile([BPC, SPAD], i16, name=f"i16{nm}", tag=f"i16{nm}")
            nc.vector.tensor_copy(w16[:], t3[:])
            # replicate 8x: w16r[b, u, 16a+v] = w16[b, 16u+v]
            w16r = idxpool.tile([BPC, SPAD // 16, 128], i16,
                                name=f"i16r{nm}", tag=f"i16r{nm}")
            for a in range(8):
                nc.vector.tensor_copy(w16r[:, :, 16 * a:16 * (a + 1)], w16[:])
            # pair-interleaved combined stream: pair q's block is
            # [x b2q, x b2q+1, y b2q, y b2q+1], each replicated 8x; batch
            # (2Q+R) of this branch lands at 8*(2*PCH*Q + SPAD*R) + branch_off
            boff2 = 0 if nm == "x" else 8 * PCH
            nc.sync.dma_start(
                bass.AP(d.tensor, doff + boff2,
                        [[16 * PCH, BPC // 2], [8 * SPAD, 2], [1, 8 * SPAD]]),
                w16r[:],
            )
        iw = idxpool.tile([128, NIW], i16, name="iw", tag="iw")
        # first 64 columns cover pair 0's fused idx block: transpose them
        # separately so the first gather's descriptor-gen starts early
        nc.sync.dma_start_transpose(
            iw[:, 0:64], bass.AP(d.tensor, doff, [[128, 64], [1, 128]])
        )
        nc.sync.dma_start_transpose(
            iw[:, 64:NIW],
            bass.AP(d.tensor, doff + 64 * 128, [[128, NIW - 64], [1, 128]]),
        )

        # PE warm-up: harmless matmuls so the PE p-state ramps while the
        # first gather chunks land (~25us of head before feats arrive)
        wmt = ps1.tile([128, S2], f32, name="warm", tag="ps1_0")
        for _ in range(100):
            nc.tensor.matmul(wmt[:], ones_row[:, 0:128], ones_row[:],
                             start=True, stop=True)

        # ---------------- feature gathers (DMA does the sampling) -----------
        # one fused gather per pair: 448 x-idxs + 448 y-idxs = 896, so the
        # first pair waits on a single Q7 descriptor-gen instead of two
        x_ap = bass.AP(x_t, 0, [[C, NP], [1, C]])
        f_tiles = {}

        def emit_gather(q):
            if q in f_tiles or q >= PAIRS:
                return
            ft = fbpool.tile([128, KC1 // 2, 2 * PCH], bf16,
                             name=f"f{q}", tag="fb")
            nc.gpsimd.dma_gather(
                ft[:], x_ap,
                iw[:, q * (PCH // 8):(q + 1) * (PCH // 8)],
                2 * PCH, 2 * PCH, C, transpose=True,
            )
            f_tiles[q] = ft

        emit_gather(0)
        emit_gather(1)

        # ---------------- weights: host-cast bf16, loaded directly ----------
        w1b = []
        for k in range(KC1):
            wb = wpool.tile([128, HID], bf16, name=f"w1b{k}", tag=f"w1b{k}")
            nc.scalar.dma_start(wb[:], w1_t[k * 128:(k + 1) * 128, :])
            w1b.append(wb)
            if k == 5:
                emit_gather(2)
        w2b = []
        for k in range(MJ):
            wb = wpool.tile([128, HID], bf16, name=f"w2b{k}", tag=f"w2b{k}")
            nc.scalar.dma_start(wb[:], w2_t[k * 128:(k + 1) * 128, :])
            w2b.append(wb)
        w3b = []
        for k in range(MJ):
            wb = wpool.tile([128, OUT], bf16, name=f"w3b{k}", tag=f"w3b{k}")
            nc.scalar.dma_start(wb[:], w3_t[k * 128:(k + 1) * 128, :])
            w3b.append(wb)
        b1c, b2c = [], []
        for j in range(MJ):
            t1 = wpool.tile([128, 1], f32, name=f"b1c{j}", tag=f"b1c{j}")
            nc.scalar.dma_start(t1[:], b1_t[j * 128:(j + 1) * 128])
            b1c.append(t1)
            t2 = wpool.tile([128, 1], f32, name=f"b2c{j}", tag=f"b2c{j}")
            nc.scalar.dma_start(t2[:], b2_t[j * 128:(j + 1) * 128])
            b2c.append(t2)
        b3c = wpool.tile([OUT, 1], f32, name="b3c", tag="b3c")
        nc.scalar.dma_start(b3c[:], b3_t[:])


        # ---------------- deltaxy ------------------------------------------
        dsub = idxpool.tile([BPC, S2], i32, name="dsub", tag="dsub")
        nc.vector.tensor_tensor(dsub[:], pxs_sb[:, 0:S2], pys_sb[:, 0:S2],
                                ALU.subtract)
        ddel = idxpool.tile([BPC, S2], f32, name="ddel", tag="ddel")
        nc.vector.tensor_scalar(ddel[:], dsub[:], float(H - 1), None, op0=ALU.add)
        nc.sync.dma_start(bass.AP(delta_t, 0, [[S2, BPC], [1, S2]]), ddel[:])

        pred_q = {}

        # ---------------- per-pair stages ----------------------------------
        h1_sb = {}       # pair -> [4 tiles [128, S2] bf16]
        h2_sb = {}

        def pair_rhs(ft, k, off):
            """[128, 2, S] strided view of one pair's columns (skips pads)."""
            base = ft[:, k, off:off + S]
            return bass.AP(base.tensor, base.offset,
                           [base.ap[0], [SPAD, 2], [1, S]])

        def emit_g1(p):
            """GEMM1 for pair p: h1T = relu(W1a^T fx + W1b^T fy + b1)."""
            ft = f_tiles.pop(p)
            h1t = [
                h1pool.tile([128, S2], bf16, name=f"h1p{p}_{j}", tag=f"h1p{j}")
                for j in range(MJ)
            ]
            for f in range(MJ):
                hp = ps1.tile([128, S2], f32, name=f"h1ps{f}_{p}",
                              tag=f"ps1_{f}")
                for b in range(2):
                    xo = b * SPAD
                    yo = PCH + b * SPAD
                    for k in range(KC1 // 2):
                        nc.tensor.matmul(
                            hp[:, b * S:(b + 1) * S],
                            w1b[k][:, f * 128:(f + 1) * 128],
                            ft[:, k, xo:xo + S],
                            start=(k == 0), stop=False,
                        )
                    for k in range(KC1 // 2):
                        nc.tensor.matmul(
                            hp[:, b * S:(b + 1) * S],
                            w1b[KC1 // 2 + k][:, f * 128:(f + 1) * 128],
                            ft[:, k, yo:yo + S],
                            start=False, stop=(k == KC1 // 2 - 1),
                        )
                if f % 2 == 0:
                    nc.scalar.activation(h1t[f][:], hp[:], AF.Relu,
                                         bias=b1c[f][:])
                else:
                    nc.vector.tensor_scalar(
                        h1t[f][:], hp[:], b1c[f][:], 0.0,
                        op0=ALU.add, op1=ALU.max,
                    )
            h1_sb[p] = h1t

        def emit_g2(p):
            """GEMM2 for pair p: h2T = relu(W2^T h1T + b2)."""
            h1t = h1_sb.pop(p)
            h2t = []
            for j in range(MJ):
                hp = ps2.tile([128, S2], f32, name=f"h2ps{j}_{p}",
                              tag=f"ps2_{j}")
                for k in range(MJ):
                    nc.tensor.matmul(
                        hp[:],
                        w2b[k][:, j * 128:(j + 1) * 128],
                        h1t[k][:],
                        start=(k == 0), stop=(k == MJ - 1),
                    )
                h2 = h2pool.tile([128, S2], bf16, name=f"h2sb{j}_{p}",
                                 tag=f"h2sb{j}")
                if j % 2 == 0:
                    nc.scalar.activation(h2[:], hp[:], AF.Relu, bias=b2c[j][:])
                else:
                    nc.vector.tensor_scalar(
                        h2[:], hp[:], b2c[j][:], 0.0, op0=ALU.add, op1=ALU.max
                    )
                h2t.append(h2)
            h2_sb[p] = h2t

        def emit_g3(p):
            """GEMM3 for pair p: predT = W3^T h2T + b3, staged per quarter."""
            q, r = divmod(p, 4)
            if r == 0:
                pred_q[q] = opool.tile([OUT, 4 * S2], f32,
                                       name=f"predq{q}", tag="predq")
            h2t = h2_sb.pop(p)
            pp = ps1.tile([OUT, S2], f32, name=f"predps_{p}", tag="ps1_0")
            for k in range(MJ):
                nc.tensor.matmul(
                    pp[:], w3b[k][:], h2t[k][:], start=(k == 0), stop=(k == MJ - 1)
                )
            nc.vector.tensor_scalar(
                pred_q[q][:, r * S2:(r + 1) * S2], pp[:], b3c[:], None,
                op0=ALU.add,
            )
            if r == 3:
                nc.sync.dma_start(
                    pred_t[:, q * 4 * S2:(q + 1) * 4 * S2],
                    pred_q.pop(q)[:],
                )

        # ---------------- main loop: software-pipelined by two pairs --------
        for p in range(PAIRS):
            emit_g1(p)
            emit_gather(p + 3)
            if p >= 1:
                emit_g2(p - 1)
            if p >= 2:
                emit_g3(p - 2)
        emit_g3(PAIRS - 2)
        emit_g2(PAIRS - 1)
        emit_g3(PAIRS - 1)

    nc.finalize()
    return nc


_NC = None


def _get_nc():
    global _NC
    if _NC is None:
        _install_ntff_hook()
        _NC = _build_nc()
    return _NC


def _make_in_maps(inputs):
    import ml_dtypes
    x = np.asarray(inputs["x"], dtype=np.float32).reshape(B, C, S)
    pxs = np.asarray(inputs["pxs"], dtype=np.int32).reshape(B, S2)
    pys = np.asarray(inputs["pys"], dtype=np.int32).reshape(B, S2)
    W1 = np.asarray(np.asarray(inputs["W1"], dtype=np.float32),
                    dtype=ml_dtypes.bfloat16)
    W2 = np.asarray(np.asarray(inputs["W2"], dtype=np.float32),
                    dtype=ml_dtypes.bfloat16)
    W3 = np.asarray(np.asarray(inputs["W3"], dtype=np.float32),
                    dtype=ml_dtypes.bfloat16)
    b1 = np.asarray(inputs["b1"], dtype=np.float32)
    b2 = np.asarray(inputs["b2"], dtype=np.float32)
    b3 = np.asarray(inputs["b3"], dtype=np.float32)
    boff = (S * np.arange(BPC)).astype(np.float32)
    in_maps = []
    for c in range(N_CORES):
        sl = slice(c * BPC, (c + 1) * BPC)
        # [BPC, C, S] -> [BPC*S, C] row-major bf16 (one contiguous row per
        # (image, position) for the transposed gather)
        xc = np.ascontiguousarray(
            x[sl].transpose(0, 2, 1).reshape(NP, C).astype(ml_dtypes.bfloat16)
        )
        in_maps.append({
            "x": xc,
            "pxs": np.ascontiguousarray(pxs[sl]),
            "pys": np.ascontiguousarray(pys[sl]),
            "W1": W1, "W2": W2, "W3": W3,
            "b1": b1, "b2": b2, "b3": b3,
            "boff": boff,
        })
    return in_maps


def _run(inputs, trace=False):
    from concourse.bass_utils import run_bass_kernel_spmd

    nc = _get_nc()
    in_maps = _make_in_maps(inputs)
    res = run_bass_kernel_spmd(
        nc, in_maps, core_ids=list(range(N_CORES)), trace=trace
    )
    pred = np.concatenate(
        [np.ascontiguousarray(res.results[c]["predT"].T) for c in range(N_CORES)],
        axis=0,
    )
    delta = np.concatenate(
        [res.results[c]["deltaxy"] for c in range(N_CORES)], axis=0
    )
    return (pred, delta), res


def kernel(**inputs):
    (pred, delta), _ = _run(inputs, trace=False)
    return pred, delta


# revision 68
# speedup vs baseline: 1.0063x; 1.0063x over previous
"""Trainium2 Bass kernel for nn_DenseRelativeLoc.

Data-parallel over batch: 32 images per NeuronCore x 8 cores.

Key idea vs the projection+one-hot baseline: the point sampling is done by
the DMA engines (dma_gather transpose mode), not the PE. Host stores x as
row-major [BPC*196, 768] bf16 in DRAM; on-device int16 row indices
(196*b + 14*px0 + px1) drive a transposed gather that lands features
directly in matmul-ready layout [128 c_in, 6 c_chunk, n_samples]. The PE
then only runs dense GEMMs:

    h1T = relu(W1a^T fx + W1b^T fy + b1)   [512, 392/pair]  (12 K-chunks)
    h2T = relu(W2^T h1T + b2)              [512, 392]
    predT = W3^T h2T + b3                  [2, 392]

All matmuls bf16 with fp32 PSUM accumulation, N=392 streams (no partition
waste, weight loads hidden under streams). Gather chunks start at even-pair
sample offsets (multiple of 16 for the wrapped idx layout) and overlap so
every pair lives contiguously in one chunk tile.

deltaxy = float(pxs - pys) + 13 computed on-device with DVE ops.
"""
import sys
import types

import numpy as np

B, C, H, W_IMG = 256, 768, 14, 14
S = 196          # sample count == H*W
HID = 512
OUT = 2
N_CORES = 8
BPC = B // N_CORES      # batches per core
NP = BPC * S            # 6272 samples per core
S2 = 2 * S              # samples per pair of batches (392)
PAIRS = BPC // 2        # 16
KC1 = 2 * C // 128      # 12 contraction chunks for GEMM1
MJ = HID // 128         # 4 HID chunks
SPAD = 224              # samples per batch padded to 14*16 in the idx stream
FPB = SPAD * BPC        # padded idx stream length (7168 = 16*448)
NIW = 2 * FPB // 16     # wrapped idx columns (896): per-pair x+y blocks
PCH = 2 * SPAD          # columns per pair per branch block (448)


def _install_ntff_hook():
    try:
        import antenv.axon_hooks  # noqa: F401
        return
    except ImportError:
        pass
    try:
        from trn_agent_boot.trn_boot import _ntff_profile_via_ctypes
        hook = _ntff_profile_via_ctypes("/opt/axon/libaxon_pjrt.so")
    except Exception:
        hook = None
    mod = types.ModuleType("antenv.axon_hooks")
    mod.get_axon_ntff_profile_hook = lambda: hook
    sys.modules["antenv.axon_hooks"] = mod


def _build_nc():
    from contextlib import ExitStack

    import concourse.bass as bass
    import concourse.bacc as bacc
    import concourse.mybir as mybir
    import concourse.tile as tile

    dt = mybir.dt
    f32, bf16, i32, i16 = dt.float32, dt.bfloat16, dt.int32, dt.int16
    AF = mybir.ActivationFunctionType
    ALU = mybir.AluOpType

    nc = bacc.Bacc(None, target_bir_lowering=False)

    # x host-prepped as [NP, C] bf16 row-major (one 1536B row per (b, hw))
    x_t = nc.dram_tensor("x", [NP, C], bf16, kind="ExternalInput")
    pxs_t = nc.dram_tensor("pxs", [BPC, S2], i32, kind="ExternalInput")
    pys_t = nc.dram_tensor("pys", [BPC, S2], i32, kind="ExternalInput")
    w1_t = nc.dram_tensor("W1", [2 * C, HID], bf16, kind="ExternalInput")
    w2_t = nc.dram_tensor("W2", [HID, HID], bf16, kind="ExternalInput")
    w3_t = nc.dram_tensor("W3", [HID, OUT], bf16, kind="ExternalInput")
    b1_t = nc.dram_tensor("b1", [HID], f32, kind="ExternalInput")
    b2_t = nc.dram_tensor("b2", [HID], f32, kind="ExternalInput")
    b3_t = nc.dram_tensor("b3", [OUT], f32, kind="ExternalInput")
    boff_t = nc.dram_tensor("boff", [BPC], f32, kind="ExternalInput")
    # predT: [2, NP] (component-major); host transposes after gather
    pred_t = nc.dram_tensor("predT", [OUT, NP], f32, kind="ExternalOutput")
    delta_t = nc.dram_tensor("deltaxy", [NP, OUT], f32, kind="ExternalOutput")

    with ExitStack() as ctx:
        tc = ctx.enter_context(tile.TileContext(nc))
        wpool = ctx.enter_context(tc.tile_pool(name="w", bufs=1))
        fbpool = ctx.enter_context(tc.tile_pool(name="fb", bufs=3))
        h1pool = ctx.enter_context(tc.tile_pool(name="h1", bufs=2))
        h2pool = ctx.enter_context(tc.tile_pool(name="h2", bufs=2))
        opool = ctx.enter_context(tc.tile_pool(name="op", bufs=2))
        idxpool = ctx.enter_context(tc.tile_pool(name="idx", bufs=1))
        ps1 = ctx.enter_context(tc.tile_pool(name="ps1", bufs=1, space="PSUM"))
        ps2 = ctx.enter_context(tc.tile_pool(name="ps2", bufs=1, space="PSUM"))
        dram = ctx.enter_context(tc.tile_pool(name="dram", bufs=1, space="DRAM"))

        # ---------------- index prep (feeds the gathers) --------------------
        # per-batch idx streams padded to SPAD=224 (a multiple of 16) so the
        # wrapped layout dma_gather wants is expressible with large-run DMAs:
        # replicate 8x in SBUF -> one contiguous write to DRAM -> one XBAR
        # transpose back as [128, NIW]
        pxs_sb = idxpool.tile([BPC, 2 * SPAD], i32, name="pxs_sb", tag="pxs_sb")
        nc.vector.memset(pxs_sb[:], 0)
        nc.sync.dma_start(pxs_sb[:, 0:S2], pxs_t[:, :])
        pys_sb = idxpool.tile([BPC, 2 * SPAD], i32, name="pys_sb", tag="pys_sb")
        nc.vector.memset(pys_sb[:], 0)
        nc.sync.dma_start(pys_sb[:, 0:S2], pys_t[:, :])
        boff_sb = idxpool.tile([BPC, 1], f32, name="boff_sb", tag="boff_sb")
        nc.sync.dma_start(boff_sb[:], boff_t[0:BPC])
        ones_row = wpool.tile([1, S2], bf16, name="ones_row", tag="ones_row")
        nc.vector.memset(ones_row[:], 1.0)

        d = dram.tile([16 * FPB], i16, name="idxd", tag="idxd")
        doff = d[:].offset
        for nm, src in (("x", pxs_sb), ("y", pys_sb)):
            tmp = idxpool.tile([BPC, SPAD], i32, name=f"itmp{nm}", tag=f"itmp{nm}")
            nc.vector.tensor_scalar(
                tmp[:], src[:, 0:2 * SPAD:2], float(W_IMG), None, op0=ALU.mult
            )
            t2 = idxpool.tile([BPC, SPAD], i32, name=f"iadd{nm}", tag=f"iadd{nm}")
            nc.vector.tensor_tensor(t2[:], tmp[:], src[:, 1:2 * SPAD:2], ALU.add)
            t3 = idxpool.tile([BPC, SPAD], f32, name=f"iglob{nm}", tag=f"iglob{nm}")
            nc.vector.tensor_scalar(
                t3[:], t2[:], boff_sb[:], None, op0=ALU.add
            )
            w16 = idxpool.tile([BPC, SPAD], i16, name=f"i16{nm}", tag=f"i16{nm}")
            nc.vector.tensor_copy(w16[:], t3[:])
            # replicate 8x: w16r[b, u, 16a+v] = w16[b, 16u+v]
            w16r = idxpool.tile([BPC, SPAD // 16, 128], i16,
                                name=f"i16r{nm}", tag=f"i16r{nm}")
            for a in range(8):
                nc.vector.tensor_copy(w16r[:, :, 16 * a:16 * (a + 1)], w16[:])
            # pair-interleaved combined stream: pair q's block is
            # [x b2q, x b2q+1, y b2q, y b2q+1], each replicated 8x; batch
            # (2Q+R) of this branch lands at 8*(2*PCH*Q + SPAD*R) + branch_off
            boff2 = 0 if nm == "x" else 8 * PCH
            nc.sync.dma_start(
                bass.AP(d.tensor, doff + boff2,
                        [[16 * PCH, BPC // 2], [8 * SPAD, 2], [1, 8 * SPAD]]),
                w16r[:],
            )
        iw = idxpool.tile([128, NIW], i16, name="iw", tag="iw")
        nc.sync.dma_start_transpose(
            iw[:], bass.AP(d.tensor, doff, [[128, NIW], [1, 128]])
        )

        # PE warm-up: harmless matmuls so the PE p-state ramps while the
        # first gather chunks land (~25us of head before feats arrive)
        wmt = ps1.tile([128, S2], f32, name="warm", tag="ps1_0")
        for _ in range(112):
            nc.tensor.matmul(wmt[:], ones_row[:, 0:128], ones_row[:],
                             start=True, stop=True)

        # ---------------- feature gathers (DMA does the sampling) -----------
        # one fused gather per pair: 448 x-idxs + 448 y-idxs = 896, so the
        # first pair waits on a single Q7 descriptor-gen instead of two
        x_ap = bass.AP(x_t, 0, [[C, NP], [1, C]])
        f_tiles = {}

        def emit_gather(q):
            if q in f_tiles or q >= PAIRS:
                return
            ft = fbpool.tile([128, KC1 // 2, 2 * PCH], bf16,
                             name=f"f{q}", tag="fb")
            nc.gpsimd.dma_gather(
                ft[:], x_ap,
                iw[:, q * (PCH // 8):(q + 1) * (PCH // 8)],
                2 * PCH, 2 * PCH, C, transpose=True,
            )
            f_tiles[q] = ft

        emit_gather(0)
        emit_gather(1)

        # ---------------- weights: host-cast bf16, loaded directly ----------
        w1b = []
        for k in range(KC1):
            wb = wpool.tile([128, HID], bf16, name=f"w1b{k}", tag=f"w1b{k}")
            nc.scalar.dma_start(wb[:], w1_t[k * 128:(k + 1) * 128, :])
            w1b.append(wb)
            if k == 5:
                emit_gather(2)
        w2b = []
        for k in range(MJ):
            wb = wpool.tile([128, HID], bf16, name=f"w2b{k}", tag=f"w2b{k}")
            nc.scalar.dma_start(wb[:], w2_t[k * 128:(k + 1) * 128, :])
            w2b.append(wb)
        w3b = []
        for k in range(MJ):
            wb = wpool.tile([128, OUT], bf16, name=f"w3b{k}", tag=f"w3b{k}")
            nc.scalar.dma_start(wb[:], w3_t[k * 128:(k + 1) * 128, :])
            w3b.append(wb)
        b1c, b2c = [], []
        for j in range(MJ):
            t1 = wpool.tile([128, 1], f32, name=f"b1c{j}", tag=f"b1c{j}")
            nc.scalar.dma_start(t1[:], b1_t[j * 128:(j + 1) * 128])
            b1c.append(t1)
            t2 = wpool.tile([128, 1], f32, name=f"b2c{j}", tag=f"b2c{j}")
            nc.scalar.dma_start(t2[:], b2_t[j * 128:(j + 1) * 128])
            b2c.append(t2)
        b3c = wpool.tile([OUT, 1], f32, name="b3c", tag="b3c")
        nc.scalar.dma_start(b3c[:], b3_t[:])


        # ---------------- deltaxy ------------------------------------------
        dsub = idxpool.tile([BPC, S2], i32, name="dsub", tag="dsub")
        nc.vector.tensor_tensor(dsub[:], pxs_sb[:, 0:S2], pys_sb[:, 0:S2],
                                ALU.subtract)
        ddel = idxpool.tile([BPC, S2], f32, name="ddel", tag="ddel")
        nc.vector.tensor_scalar(ddel[:], dsub[:], float(H - 1), None, op0=ALU.add)
        nc.sync.dma_start(bass.AP(delta_t, 0, [[S2, BPC], [1, S2]]), ddel[:])

        pred_q = {}

        # ---------------- per-pair stages ----------------------------------
        h1_sb = {}       # pair -> [4 tiles [128, S2] bf16]
        h2_sb = {}

        def pair_rhs(ft, k, off):
            """[128, 2, S] strided view of one pair's columns (skips pads)."""
            base = ft[:, k, off:off + S]
            return bass.AP(base.tensor, base.offset,
                           [base.ap[0], [SPAD, 2], [1, S]])

        def emit_g1(p):
            """GEMM1 for pair p: h1T = relu(W1a^T fx + W1b^T fy + b1)."""
            ft = f_tiles.pop(p)
            h1t = [
                h1pool.tile([128, S2], bf16, name=f"h1p{p}_{j}", tag=f"h1p{j}")
                for j in range(MJ)
            ]
            for f in range(MJ):
                hp = ps1.tile([128, S2], f32, name=f"h1ps{f}_{p}",
                              tag=f"ps1_{f}")
                for b in range(2):
                    xo = b * SPAD
                    yo = PCH + b * SPAD
                    for k in range(KC1 // 2):
                        nc.tensor.matmul(
                            hp[:, b * S:(b + 1) * S],
                            w1b[k][:, f * 128:(f + 1) * 128],
                            ft[:, k, xo:xo + S],
                            start=(k == 0), stop=False,
                        )
                    for k in range(KC1 // 2):
                        nc.tensor.matmul(
                            hp[:, b * S:(b + 1) * S],
                            w1b[KC1 // 2 + k][:, f * 128:(f + 1) * 128],
                            ft[:, k, yo:yo + S],
                            start=False, stop=(k == KC1 // 2 - 1),
                        )
                if f % 2 == 0:
                    nc.scalar.activation(h1t[f][:], hp[:], AF.Relu,
                                         bias=b1c[f][:])
                else:
                    nc.vector.tensor_scalar(
                        h1t[f][:], hp[:], b1c[f][:], 0.0,
                        op0=ALU.add, op1=ALU.max,
                    )
            h1_sb[p] = h1t

        def emit_g2(p):
            """GEMM2 for pair p: h2T = relu(W2^T h1T + b2)."""
            h1t = h1_sb.pop(p)
            h2t = []
            for j in range(MJ):
                hp = ps2.tile([128, S2], f32, name=f"h2ps{j}_{p}",
                              tag=f"ps2_{j}")
                for k in range(MJ):
                    nc.tensor.matmul(
                        hp[:],
                        w2b[k][:, j * 128:(j + 1) * 128],
                        h1t[k][:],
                        start=(k == 0), stop=(k == MJ - 1),
                    )
                h2 = h2pool.tile([128, S2], bf16, name=f"h2sb{j}_{p}",
                                 tag=f"h2sb{j}")
                if j % 2 == 0:
                    nc.scalar.activation(h2[:], hp[:], AF.Relu, bias=b2c[j][:])
                else:
                    nc.vector.tensor_scalar(
                        h2[:], hp[:], b2c[j][:], 0.0, op0=ALU.add, op1=ALU.max
                    )
                h2t.append(h2)
            h2_sb[p] = h2t

        def emit_g3(p):
            """GEMM3 for pair p: predT = W3^T h2T + b3, staged per quarter."""
            q, r = divmod(p, 4)
            if r == 0:
                pred_q[q] = opool.tile([OUT, 4 * S2], f32,
                                       name=f"predq{q}", tag="predq")
            h2t = h2_sb.pop(p)
            pp = ps1.tile([OUT, S2], f32, name=f"predps_{p}", tag="ps1_0")
            for k in range(MJ):
                nc.tensor.matmul(
                    pp[:], w3b[k][:], h2t[k][:], start=(k == 0), stop=(k == MJ - 1)
                )
            nc.vector.tensor_scalar(
                pred_q[q][:, r * S2:(r + 1) * S2], pp[:], b3c[:], None,
                op0=ALU.add,
            )
            if r == 3:
                nc.sync.dma_start(
                    pred_t[:, q * 4 * S2:(q + 1) * 4 * S2],
                    pred_q.pop(q)[:],
                )

        # ---------------- main loop: software-pipelined by two pairs --------
        for p in range(PAIRS):
            emit_g1(p)
            emit_gather(p + 3)
            if p >= 1:
                emit_g2(p - 1)
            if p >= 2:
                emit_g3(p - 2)
        emit_g3(PAIRS - 2)
        emit_g2(PAIRS - 1)
        emit_g3(PAIRS - 1)

    nc.finalize()
    return nc


_NC = None


def _get_nc():
    global _NC
    if _NC is None:
        _install_ntff_hook()
        _NC = _build_nc()
    return _NC


def _make_in_maps(inputs):
    import ml_dtypes
    x = np.asarray(inputs["x"], dtype=np.float32).reshape(B, C, S)
    pxs = np.asarray(inputs["pxs"], dtype=np.int32).reshape(B, S2)
    pys = np.asarray(inputs["pys"], dtype=np.int32).reshape(B, S2)
    W1 = np.asarray(np.asarray(inputs["W1"], dtype=np.float32),
                    dtype=ml_dtypes.bfloat16)
    W2 = np.asarray(np.asarray(inputs["W2"], dtype=np.float32),
                    dtype=ml_dtypes.bfloat16)
    W3 = np.asarray(np.asarray(inputs["W3"], dtype=np.float32),
                    dtype=ml_dtypes.bfloat16)
    b1 = np.asarray(inputs["b1"], dtype=np.float32)
    b2 = np.asarray(inputs["b2"], dtype=np.float32)
    b3 = np.asarray(inputs["b3"], dtype=np.float32)
    boff = (S * np.arange(BPC)).astype(np.float32)
    in_maps = []
    for c in range(N_CORES):
        sl = slice(c * BPC, (c + 1) * BPC)
        # [BPC, C, S] -> [BPC*S, C] row-major bf16 (one contiguous row per
        # (image, position) for the transposed gather)
        xc = np.ascontiguousarray(
            x[sl].transpose(0, 2, 1).reshape(NP, C).astype(ml_dtypes.bfloat16)
        )
        in_maps.append({
            "x": xc,
            "pxs": np.ascontiguousarray(pxs[sl]),
            "pys": np.ascontiguousarray(pys[sl]),
            "W1": W1, "W2": W2, "W3": W3,
            "b1": b1, "b2": b2, "b3": b3,
            "boff": boff,
        })
    return in_maps


def _run(inputs, trace=False):
    from concourse.bass_utils import run_bass_kernel_spmd

    nc = _get_nc()
    in_maps = _make_in_maps(inputs)
    res = run_bass_kernel_spmd(
        nc, in_maps, core_ids=list(range(N_CORES)), trace=trace
    )
    pred = np.concatenate(
        [np.ascontiguousarray(res.results[c]["predT"].T) for c in range(N_CORES)],
        axis=0,
    )
    delta = np.concatenate(
        [res.results[c]["deltaxy"] for c in range(N_CORES)], axis=0
    )
    return (pred, delta), res


def kernel(**inputs):
    (pred, delta), _ = _run(inputs, trace=False)
    return pred, delta
